# revision 1
# baseline (speedup 1.0000x reference)
"""Trainium2 Bass kernel for GQA sliding-window attention (8-core SPMD).

Problem: B=8, S=32, D=4096, H=32 Q-heads, KVH=8 KV-heads, HD=128,
sliding window 4096 with 4064 cached positions.

Sharding: tensor-parallel over heads. Core c owns Q heads 4c..4c+3 and KV
head c (one GQA group): Wq/Wk/Wv column-sharded, cache sharded by KV head,
x replicated. Each core computes its 4 heads' attention output in two
head-pair passes; after each pass the cores AllGather that pass's (bf16)
attention outputs so the gather overlaps the next pass's compute. Each core
then applies a column slice of Wo, and the host concatenates column slices
(no host-side arithmetic).

Compute is bf16 on the TensorEngine (fp32 PSUM accumulation, fp32 softmax
intermediates). Host-side sharding also does layout prep:
  - x is fed transposed (xT [D, 256]) so QKV projections produce Q^T/K^T
    directly in [head_dim, token] layout.
  - Wq/Wk columns (and cached K's hd axis) are permuted so RoPE's interleaved
    (even,odd) pairs become contiguous halves [0:64]=real, [64:128]=imag.
    The permutation cancels in q.k since both sides share it; V/Wo stay
    unpermuted.
  - SCALE = HD^-0.5 is folded into Wq.

Softmax skips max-subtraction (scores are O(10), exp is safe in fp32) and
normalization is deferred: unnormalized exp(scores) feeds attn@V, row sums
come from a ones-vector matmul, and 1/sum is applied when copying the
attention output out of PSUM.
"""

import os
import sys
from contextlib import ExitStack

import numpy as np
import ml_dtypes

import concourse.bass as bass
import concourse.tile as tile
import concourse.mybir as mybir
from concourse import bacc
from concourse.bass_utils import run_bass_kernel_spmd
from concourse.masks import make_identity

BF16 = ml_dtypes.bfloat16

CORES = 8
B, S, D = 8, 32, 4096
H, KVH, HD = 32, 8, 128
SW = 4096
PREV = SW - S  # 4064
TOK = B * S  # 256
NH = H // KVH  # 4 Q heads per core
NHP = NH // 2  # head pairs per core
QCOLS = NH * HD  # 512 Q-projection columns per core
SCALE = float(HD) ** -0.5

# hd permutation: interleaved (r0,i0,r1,i1,...) -> (r..., i...)
_IDX = np.concatenate([np.arange(0, HD, 2), np.arange(1, HD, 2)])

# exec time of the last traced run (ns), set when KERNEL_TRACE=1
LAST_EXEC_NS = None

_BUILD_CACHE = {}


def _install_ntff_hook():
    """Register the axon NTFF profiling hook (the agent image's antenv stub
    lacks axon_hooks). Only needed when tracing."""
    import types

    if "antenv.axon_hooks" in sys.modules:
        return
    try:
        from trn_agent_boot.trn_boot import _ntff_profile_via_ctypes

        hook = _ntff_profile_via_ctypes("/opt/axon/libaxon_pjrt.so")
    except Exception:
        hook = None
    mod = types.ModuleType("antenv.axon_hooks")
    mod._hook = hook
    mod.get_axon_ntff_profile_hook = lambda: mod._hook
    mod.set_axon_ntff_profile_hook = lambda h: setattr(mod, "_hook", h)
    sys.modules["antenv.axon_hooks"] = mod
    import antenv

    antenv.axon_hooks = mod


def build(d=D, prev=PREV, cores=CORES):
    """Build the per-core Bass graph. d = model dim, prev = cached positions
    (both parameterizable for cheap simulation)."""
    assert d % 128 == 0 and d % cores == 0
    n_dc = d // 128  # contraction chunks for QKV projections
    n_hc = (H * HD) // 128  # contraction chunks for Wo (fixed head structure)
    outc = d // cores  # output columns per core (Wo column slice)
    n_tc = (prev + 127) // 128  # cache t-chunks (last may be short)
    tail = prev - (n_tc - 1) * 128  # rows in last chunk
    assert 0 < tail <= 128
    n_xp = max(1, n_dc // 4)  # xt/wq DMA pieces (fine-grained deps)
    xp = n_dc // n_xp

    dt = mybir.dt
    bf, f32 = dt.bfloat16, dt.float32
    EXP = mybir.ActivationFunctionType.Exp

    nc = bacc.Bacc("TRN2", target_bir_lowering=False, debug=False, num_devices=cores)

    xt_d = nc.dram_tensor("xt", [d, TOK], bf, kind="ExternalInput")
    wq_d = nc.dram_tensor("wq", [d, QCOLS], bf, kind="ExternalInput")
    wkv_d = nc.dram_tensor("wkv", [d, 2 * HD], bf, kind="ExternalInput")
    kct_d = nc.dram_tensor("kct", [HD, prev], bf, kind="ExternalInput")
    vc_d = nc.dram_tensor("vc", [prev, HD], bf, kind="ExternalInput")
    wo_d = nc.dram_tensor("wo", [H * HD, outc], bf, kind="ExternalInput")
    cost_d = nc.dram_tensor("cost", [HD // 2, TOK], f32, kind="ExternalInput")
    sint_d = nc.dram_tensor("sint", [HD // 2, TOK], f32, kind="ExternalInput")
    maskt_d = nc.dram_tensor("maskt", [S, TOK], f32, kind="ExternalInput")
    out_d = nc.dram_tensor("out", [TOK, outc], f32, kind="ExternalOutput")

    with tile.TileContext(nc) as tc, ExitStack() as ctx:
        from concourse.tile import add_dep_helper

        const = ctx.enter_context(tc.tile_pool(name="const", bufs=1))

        xt_sb = [const.tile([128, xp, TOK], bf, tag=f"xt{i}", name=f"xt{i}") for i in range(n_xp)]
        wq_sb = [const.tile([128, xp, QCOLS], bf, tag=f"wqp{i}", name=f"wqp{i}") for i in range(n_xp)]
        wkv_sb = const.tile([128, n_dc, 2 * HD], bf)
        kct_sb = const.tile([128, prev], bf)
        vc_sb = const.tile([128, n_tc, HD], bf)
        wo_sb = const.tile([128, n_hc, outc], bf)
        cost_sb = const.tile([HD // 2, TOK], f32)
        sint_sb = const.tile([HD // 2, TOK], f32)
        maskt_sb = const.tile([S, B, S], f32)
        ones_sb = const.tile([128, 1], bf)
        ident_sb = const.tile([128, 128], bf)
        qT_sb = [
            const.tile([128, 2, TOK], bf, tag=f"qT{p}", name=f"qT{p}")
            for p in range(NHP)
        ]
        kTn_sb = const.tile([128, TOK], bf)
        vnT_sb = const.tile([128, TOK], bf)
        vn_sb = const.tile([S, B, HD], bf)
        attn_new = [
            const.tile([S, 2, TOK], bf, tag=f"an{p}", name=f"an{p}")
            for p in range(NHP)
        ]
        recip_sb = [const.tile([1, 2 * TOK], f32, tag=f"rc{p}", name=f"rc{p}") for p in range(NHP)]
        recip_bc = [const.tile([128, 2 * TOK], f32, tag=f"rb{p}", name=f"rb{p}") for p in range(NHP)]
        attnout = [const.tile([128, 2 * TOK], bf, tag=f"ao{p}", name=f"ao{p}") for p in range(NHP)]
        all_sb = [
            [const.tile([128, 2 * TOK], bf, tag=f"all{p}_{r}", name=f"all{p}_{r}") for r in range(cores)]
            for p in range(NHP)
        ]
        out_sb = const.tile([128, 2, outc], f32, name="out_sb")

        # ---- constants built on-device ----
        nc.gpsimd.memset(ones_sb[:], 1.0)
        warm_sb = const.tile([1, 64], bf, name="warm_sb")
        nc.vector.memset(warm_sb[:], 0.0)
        make_identity(nc, ident_sb[:])
        for p in range(NHP):
            nc.vector.memset(attn_new[p][:], 0.0)

        # ---- input DMAs (sync engine = HWDGE, FIFO => streaming order).
        # Order follows first-use: xt + per-head wq pieces pace the
        # head-major projection; kct before pass-0 scores; wo last.
        nc.scalar.dma_start(out=cost_sb[:], in_=cost_d.ap())
        nc.scalar.dma_start(out=sint_sb[:], in_=sint_d.ap())
        nc.scalar.dma_start(
            out=maskt_sb[:], in_=maskt_d.ap().rearrange("p (b s) -> p b s", b=B)
        )
        xt_r = xt_d.ap().rearrange("(c p) n -> p c n", p=128)
        wq_r = wq_d.ap().rearrange("(c p) n -> p c n", p=128)
        for i in range(n_xp):
            nc.sync.dma_start(out=xt_sb[i][:], in_=xt_r[:, i * xp : (i + 1) * xp, :])
            nc.sync.dma_start(out=wq_sb[i][:], in_=wq_r[:, i * xp : (i + 1) * xp, :])
        nc.sync.dma_start(out=wkv_sb[:], in_=wkv_d.ap().rearrange("(c p) n -> p c n", p=128))
        nc.scalar.dma_start(out=kct_sb[:], in_=kct_d.ap())
        if n_tc > 1:
            nc.scalar.dma_start(
                out=vc_sb[:, 0 : n_tc - 1, :],
                in_=vc_d.ap()[0 : (n_tc - 1) * 128, :].rearrange(
                    "(c p) n -> p c n", p=128
                ),
            )
        nc.scalar.dma_start(
            out=vc_sb[0:tail, n_tc - 1, :], in_=vc_d.ap()[(n_tc - 1) * 128 : prev, :]
        )
        wo_r = wo_d.ap().rearrange("(c p) n -> p c n", p=128)
        wo_pieces = max(1, n_hc // 8)
        wo_step = n_hc // wo_pieces
        for i in range(wo_pieces):
            sl = slice(i * wo_step, (i + 1) * wo_step)
            nc.sync.dma_start(out=wo_sb[:, sl, :], in_=wo_r[:, sl, :])

        rtmp = ctx.enter_context(tc.tile_pool(name="rope_tmp", bufs=4))

        def rope(src_ps, dst):
            hh = HD // 2
            qr, qi = src_ps[0:hh, :], src_ps[hh:128, :]
            t1 = rtmp.tile([hh, TOK], f32, tag="t1", name="t1")
            t2 = rtmp.tile([hh, TOK], f32, tag="t2", name="t2")
            nc.vector.tensor_mul(t1[:], qr, cost_sb[:])
            nc.vector.tensor_mul(t2[:], qi, sint_sb[:])
            nc.vector.tensor_sub(dst[0:hh, :], t1[:], t2[:])
            t3 = rtmp.tile([hh, TOK], f32, tag="t1", name="t1")
            t4 = rtmp.tile([hh, TOK], f32, tag="t2", name="t2")
            nc.vector.tensor_mul(t3[:], qr, sint_sb[:])
            nc.vector.tensor_mul(t4[:], qi, cost_sb[:])
            nc.vector.tensor_add(dst[hh:128, :], t3[:], t4[:])

        # ---- PE warmup: ~10us of back-to-back matmuls so the HAM clock
        # gate reaches full rate before the real work arrives ----
        warm_rhs = const.tile([128, 512], bf, name="warm_rhs")
        nc.vector.memset(warm_rhs[:], 0.0)
        with tc.tile_pool(name="warm_ps", bufs=1, space="PSUM") as warm_pool:
            wps = warm_pool.tile([128, 512], f32, tag="wps", name="wps")
            for _ in range(24):
                nc.tensor.matmul(
                    wps[:], warm_rhs[:, 0:128], warm_rhs[:],
                    start=True, stop=True, skip_group_check=True,
                )

        # ---- phase 1: QKV projection, head-major so RoPE overlaps ----
        # Projection PSUM is split into two pools so the early pool's
        # release (q0,q1,v,k + their RoPE reads) unblocks the attention
        # pools without waiting for q2/q3's late RoPEs (pool releases are
        # zone-granular: a new pool's first write waits the whole freed
        # pool's release).
        def proj_mm(dst, lhs, c, st, sp):
            nc.tensor.matmul(
                dst, lhs, xt_sb[c // xp][:, c % xp, :],
                start=st, stop=sp, skip_group_check=True,
            )

        with tc.tile_pool(name="proj_psA", bufs=1, space="PSUM") as proj_a:
            qkv_tiles = {
                i: proj_a.tile([128, 512], f32, tag=f"qkv{i}", name=f"qkv{i}")
                for i in (0, 1, NH + 1, NH)  # q0, q1, v, k
            }

            def proj_q(h):
                for c in range(n_dc):
                    proj_mm(
                        qkv_tiles[h][:, 0:TOK],
                        wq_sb[c // xp][:, c % xp, h * HD : (h + 1) * HD],
                        c, c == 0, c == n_dc - 1,
                    )
                rope(qkv_tiles[h][:, 0:TOK], qT_sb[h // 2][:, h % 2, :])

            def proj_kv_slice(off, slot):
                for c in range(n_dc):
                    proj_mm(
                        qkv_tiles[slot][:, 0:TOK], wkv_sb[:, c, off : off + HD],
                        c, c == 0, c == n_dc - 1,
                    )

            proj_q(0)
            proj_q(1)
            proj_kv_slice(HD, NH + 1)
            # V_new^T -> per-batch V_new [t=32, hd] via PE transpose
            nc.scalar.copy(vnT_sb[:], qkv_tiles[NH + 1][:, 0:TOK])
            with tc.tile_pool(name="vt_ps", bufs=2, space="PSUM") as vt_pool:
                for b in range(B):
                    vt = vt_pool.tile([S, HD], bf, tag="vt", name="vt")
                    nc.tensor.transpose(vt[:], vnT_sb[:, b * S : (b + 1) * S], ident_sb[:])
                    nc.scalar.copy(vn_sb[:, b, :], vt[:])
            proj_kv_slice(0, NH)
            rope(qkv_tiles[NH][:, 0:TOK], kTn_sb)

        proj_b = ctx.enter_context(tc.tile_pool(name="proj_psB", bufs=1, space="PSUM"))
        qb_tiles = {
            h: proj_b.tile([128, 512], f32, tag=f"qkvb{h}", name=f"qkvb{h}")
            for h in (2, 3)
        }
        for h in (2, 3):
            for c in range(n_dc):
                proj_mm(
                    qb_tiles[h][:, 0:TOK],
                    wq_sb[c // xp][:, c % xp, h * HD : (h + 1) * HD],
                    c, c == 0, c == n_dc - 1,
                )
            rope(qb_tiles[h][:, 0:TOK], qT_sb[h // 2][:, h % 2, :])
        # (pass emission order handled below)

        # ---- phase 2+3: attention in two head-pair passes, AllGather each ----
        dram = ctx.enter_context(tc.tile_pool(name="dram", bufs=1, space="DRAM"))
        ag_in = [dram.tile([128, 2 * TOK], bf, tag=f"agi{p}", name=f"agi{p}") for p in range(NHP)]
        ag_out = [
            dram.tile(
                [128 * cores, 2 * TOK], bf, tag=f"ago{p}", name=f"ago{p}",
                addr_space="Shared",
            )
            for p in range(NHP)
        ]
        agw_out_sh = None
        agw_in = dram.tile([1, 64], bf, name="agw_in")
        agw_out = dram.tile([cores, 64], bf, name="agw_out", addr_space="Shared")

        # tiny early AllGather warms up the collectives firmware so the
        # first real gather's trigger->start latency is ~2us instead of ~12us
        nc.gpsimd.dma_start(out=agw_in[:], in_=warm_sb[:])
        nc.gpsimd.collective_compute(
            "AllGather",
            mybir.AluOpType.bypass,
            replica_groups=[list(range(cores))],
            ins=[agw_in.opt()],
            outs=[agw_out.opt()],
        )

        s_pool = ctx.enter_context(tc.tile_pool(name="s_ps", bufs=2, space="PSUM"))
        acc_pool = ctx.enter_context(tc.tile_pool(name="acc_ps", bufs=1, space="PSUM"))
        attn_pool = ctx.enter_context(tc.tile_pool(name="attn", bufs=4))
        wo_pool = ctx.enter_context(tc.tile_pool(name="wo_ps", bufs=1, space="PSUM"))
        out_ps = [wo_pool.tile([128, outc], f32, tag=f"out{k}", name=f"out{k}") for k in range(2)]

        norm_insts = []  # pass-0 normalize instruction, for pass serialization
        for p in range(NHP):
            h0 = 2 * p  # first head of the pair
            qpair = qT_sb[p][:, :, :]  # [128, 2, TOK]
            o_ps = acc_pool.tile([128, 2, TOK], f32, tag="o", name="o")
            sum_ps = acc_pool.tile([1, 2, TOK], f32, tag="sum", name="sum")

            # chunk loop, software-pipelined: sum/av for chunk t-1 are issued
            # after the chunk-t scores so they never wait on a fresh exp
            prev_work = None
            for t in range(n_tc):
                n = 128 if t < n_tc - 1 else tail
                s_ps = s_pool.tile([128, 2, TOK], f32, tag="s", name="s")
                mm = nc.tensor.matmul(
                    s_ps[0:n, :, :], kct_sb[:, t * 128 : t * 128 + n], qpair,
                    start=True, stop=True, skip_group_check=True,
                )
                if p == 1:
                    norm_insts.append(mm)
                a_sb = attn_pool.tile([128, 2, TOK], bf, tag="a", name="a")
                nc.scalar.activation(a_sb[0:n, :, :], s_ps[0:n, :, :], EXP)
                if prev_work is not None:
                    pa, pn, pt = prev_work
                    nc.tensor.matmul(
                        sum_ps[0:1, :, :], ones_sb[0:pn, 0:1], pa[0:pn, :, :],
                        start=(pt == 0), stop=False, skip_group_check=True,
                    )
                    nc.tensor.matmul(
                        o_ps[:, :, :], vc_sb[0:pn, pt, :], pa[0:pn, :, :],
                        start=(pt == 0), stop=False, skip_group_check=True,
                    )
                prev_work = (a_sb, n, t)
            pa, pn, pt = prev_work
            nc.tensor.matmul(
                sum_ps[0:1, :, :], ones_sb[0:pn, 0:1], pa[0:pn, :, :],
                start=(pt == 0), stop=False, skip_group_check=True,
            )
            nc.tensor.matmul(
                o_ps[:, :, :], vc_sb[0:pn, pt, :], pa[0:pn, :, :],
                start=(pt == 0), stop=False, skip_group_check=True,
            )

            # new-token part (t = prev..prev+S), per batch
            sn_ps = s_pool.tile([S, B, 2 * S], f32, tag="s", name="s")
            anp = attn_new[p][0:S, :, :]
            for b in range(B):
                nc.tensor.matmul(
                    sn_ps[0:S, b, :], kTn_sb[:, b * S : (b + 1) * S],
                    qpair[:, :, b * S : (b + 1) * S], start=True, stop=True,
                    skip_group_check=True,
                )
                nc.vector.scalar_tensor_tensor(
                    out=sn_ps[0:S, b, :].rearrange("p (h s) -> p h s", h=2),
                    in0=sn_ps[0:S, b, :].rearrange("p (h s) -> p h s", h=2),
                    scalar=0.0,
                    in1=maskt_sb[:, b, :].unsqueeze(1).broadcast_to((S, 2, S)),
                    op0=mybir.AluOpType.add,
                    op1=mybir.AluOpType.add,
                )
                nc.scalar.activation(
                    anp[:, :, b * S : (b + 1) * S],
                    sn_ps[0:S, b, :].rearrange("p (h s) -> p h s", h=2),
                    EXP,
                )
            nc.tensor.matmul(
                sum_ps[0:1, :, :], ones_sb[0:S, 0:1], anp,
                start=False, stop=True, skip_group_check=True,
            )
            for b in range(B):
                for l in range(2):
                    last_av = nc.tensor.matmul(
                        o_ps[:, l, b * S : (b + 1) * S],
                        vn_sb[0:S, b, :],
                        anp[:, l, b * S : (b + 1) * S],
                        start=False, stop=(b == B - 1 and l == 1),
                        skip_group_check=True,
                    )

            # 1/rowsum -> broadcast -> normalize on PSUM->SBUF copy
            nc.vector.reciprocal_approx_fast(
                recip_sb[p][:], sum_ps[0:1, :, :].rearrange("p h s -> p (h s)")
            )
            nc.gpsimd.partition_broadcast(recip_bc[p][:], recip_sb[p][:])
            norm = nc.vector.tensor_mul(
                attnout[p][:],
                o_ps[:, :, :].rearrange("p h s -> p (h s)"),
                recip_bc[p][:],
            )
            if p == 0:
                p0_norm = norm

            # AllGather this pass's heads (overlaps next pass's compute)
            nc.scalar.dma_start(ag_in[p][:], attnout[p][:])
            nc.gpsimd.collective_compute(
                "AllGather",
                mybir.AluOpType.bypass,
                replica_groups=[list(range(cores))],
                ins=[ag_in[p].opt()],
                outs=[ag_out[p].opt()],
            )
            ag_r = ag_out[p].rearrange("(r p) n -> p r n", p=128)
            for r in range(cores):
                nc.sync.dma_start(all_sb[p][r][:], ag_r[:, r, :])

        # keep pass-1 scores behind pass-0's normalize so pass-0's AllGather
        # launches at the midpoint and overlaps pass-1 compute
        for mm in norm_insts:
            add_dep_helper(mm.ins, p0_norm.ins, sync=True, reason="serialize passes")

        # ---- phase 4: out = attnout_all @ Wo[:, slice], per pass ----
        for p in range(NHP):
            h0 = 2 * p
            for k in range(2):
                for r in range(cores):
                    for l in range(2):
                        g = r * NH + h0 + l
                        mm = nc.tensor.matmul(
                            out_ps[k][:],
                            all_sb[p][r][:, l * TOK + k * 128 : l * TOK + k * 128 + 128],
                            wo_sb[:, g, :],
                            start=(p == 0 and r == 0 and l == 0),
                            stop=(p == NHP - 1 and r == cores - 1 and l == 1),
                            skip_group_check=True,
                        )
                        if p == 0 and r == 0 and l == 0:
                            # keep Wo behind pass-1's attention in the PE
                            # stream (the cost model underestimates the
                            # AllGather and would otherwise stall pass-1)
                            add_dep_helper(
                                mm.ins, last_av.ins, sync=True,
                                reason="Wo after pass-1 attention",
                            )

        # ---- output: PSUM -> SBUF -> DRAM ----
        out_r = out_d.ap().rearrange("(k p) n -> p k n", p=128)
        for k in range(2):
            nc.scalar.copy(out_sb[:, k, :], out_ps[k][:])
            nc.sync.dma_start(out_r[:, k, :], out_sb[:, k, :])

    nc.compile()
    return nc


def prep_in_maps(x, freqs_cos, freqs_sin, mask, cache_k, cache_v, Wq, Wk, Wv, Wo,
                 d=D, prev=PREV, cores=CORES):
    """Host-side sharding/layout. Returns in_maps for run_bass_kernel_spmd."""
    x = np.asarray(x, np.float32).reshape(TOK, d)
    xt = np.ascontiguousarray(x.T).astype(BF16)  # [d, TOK]
    cost = np.ascontiguousarray(
        np.tile(np.asarray(freqs_cos, np.float32)[0].T, (1, B))
    )  # [64, TOK]
    sint = np.ascontiguousarray(
        np.tile(np.asarray(freqs_sin, np.float32)[0].T, (1, B))
    )
    maskt = np.ascontiguousarray(
        np.asarray(mask, np.float32).transpose(2, 0, 1).reshape(S, TOK)
    )
    Wq = np.asarray(Wq, np.float32)
    Wk = np.asarray(Wk, np.float32)
    Wv = np.asarray(Wv, np.float32)
    Wo = np.asarray(Wo, np.float32)
    cache_k = np.asarray(cache_k, np.float32)
    cache_v = np.asarray(cache_v, np.float32)

    outc = d // cores
    in_maps = []
    for c in range(cores):
        wq_c = (Wq[:, c * QCOLS : (c + 1) * QCOLS] * SCALE).reshape(d, NH, HD)[
            :, :, _IDX
        ].reshape(d, QCOLS)
        wk_c = Wk[:, c * HD : (c + 1) * HD][:, _IDX]
        wv_c = Wv[:, c * HD : (c + 1) * HD]
        wkv_c = np.concatenate([wk_c, wv_c], axis=1)  # [d, 256]
        kct_c = np.ascontiguousarray(cache_k[0, :prev, c, :][:, _IDX].T)  # [HD, prev]
        vc_c = np.ascontiguousarray(cache_v[0, :prev, c, :])  # [prev, HD]
        wo_c = Wo[:, c * outc : (c + 1) * outc]
        in_maps.append(
            {
                "xt": xt,
                "wq": np.ascontiguousarray(wq_c).astype(BF16),
                "wkv": np.ascontiguousarray(wkv_c).astype(BF16),
                "kct": kct_c.astype(BF16),
                "vc": vc_c.astype(BF16),
                "wo": np.ascontiguousarray(wo_c).astype(BF16),
                "cost": cost,
                "sint": sint,
                "maskt": maskt,
            }
        )
    return in_maps


def kernel(x, freqs_cos, freqs_sin, mask, cache_k, cache_v, Wq, Wk, Wv, Wo, positions):
    global LAST_EXEC_NS
    assert int(positions) == PREV, f"kernel compiled for positions={PREV}"

    key = (D, PREV)
    if key not in _BUILD_CACHE:
        _BUILD_CACHE[key] = build(D, PREV, CORES)
    nc = _BUILD_CACHE[key]

    in_maps = prep_in_maps(
        x, freqs_cos, freqs_sin, mask, cache_k, cache_v, Wq, Wk, Wv, Wo
    )

    trace = os.environ.get("KERNEL_TRACE", "0") == "1"
    if trace:
        _install_ntff_hook()
    res = run_bass_kernel_spmd(
        nc, in_maps, core_ids=list(range(CORES)), trace=trace
    )
    if trace:
        LAST_EXEC_NS = res.exec_time_ns

    outc = D // CORES
    out = np.empty((TOK, D), np.float32)
    for c in range(CORES):
        out[:, c * outc : (c + 1) * outc] = res.results[c]["out"]
    return out.reshape(B, S, D)

